# revision 6
# baseline (speedup 1.0000x reference)
# kernel.py — Trainium2 Bass kernel for nn_KANLinear (LayerNorm -> silu/base + cubic
# B-spline/spline matmuls), data-parallel over the batch dim across 8 NeuronCores.
#
# Math (per core, x: [S=2048, F=512]):
#   xn   = LayerNorm(x) * gamma + beta          (eps = 1e-3)
#   out  = silu(xn) @ base_weight.T + bases(xn) @ spline_weight.T + base_bias
# where bases are the 6 cubic B-spline basis functions on the uniform grid
# [-3, 3] (h = 2/3). Closed form used on-device (verified exact vs Cox-de Boor):
#   u     = 1.5*xn + 4.5
#   a_j   = |u - (j+2)|                 j = 0..5
#   6*B_j = (cbrt4*min(a_j-1,0))^3 - min(a_j-2,0)^3
# The 1/6 is folded into the spline weights host-side; gamma/beta are folded
# into the per-partition scale/bias of the Abs/Silu activation passes.
#
# Engine mapping per core:
#   DVE   : bn_stats/bn_aggr LN stats, (x-mu)*rstd, 2 custom DVE cube ops per basis
#   ACT   : sqrt(var+eps), |scale*xn+bias| passes, Silu
#   PE    : 64 transposes of xn ([s,f] -> [f,s]) + 448 fp16 matmuls
#   DMA   : x in, fp16 weights in, PSUM -> DRAM out
# Output accumulates base+spline matmuls in PSUM in natural [s, u] layout.

import os
import numpy as np

import concourse.bacc as bacc
import concourse.mybir as mybir
from concourse.tile import TileContext
from concourse.bass_utils import run_bass_kernel_spmd
from concourse.masks import make_identity
from concourse.dve_spec import Spec, Src0, Src1, C0, C1, Zero, minn, sq, lower
from concourse.dve_ops import DveOp, OPS
import concourse.dve_ops as dve_ops_mod
from concourse.dve_uop import DveOpSpec

B, S, F, U, J = 8, 2048, 512, 512, 6
NCORES = 8
FT = F // 128          # 4 f-tiles
ST = S // 128          # 16 s-tiles
CHUNK = 512            # tokens per processing chunk
NCH = S // CHUNK       # 4 chunks
LT = CHUNK // 128      # 4 s-subtiles per chunk
CBRT4 = float(4.0 ** (1.0 / 3.0))
LN_EPS = 1e-3

_f32 = mybir.dt.float32
_f16 = mybir.dt.float16


def _register_ops():
    """Register the two custom DVE ops (idempotent)."""
    existing = {op.name for op in OPS}

    def add(name, spec, rd1):
        if name in existing:
            return next(op for op in OPS if op.name == name)
        shas = {}
        for ver in ("v3",):
            s = DveOpSpec(name=name, opcode=0, uops=lower(spec, ver=ver), rd1_en=rd1)
            shas[ver] = s.sha(ver)
        op = DveOp(name, spec, subdim=False, uops_sha=shas)
        OPS.append(op)
        dve_ops_mod.CUSTOM_DVE_SPECS[op.name] = op.spec
        dve_ops_mod._SUB_OPCODE_FOR_NAME[op.name] = (
            dve_ops_mod._CUSTOM_DVE_ROW_BASE + len(OPS) - 1
        )
        return op

    # out = (C1 * min(in0 - C0, 0))^3          (C0=1, C1=cbrt4  -> 4*m1^3, <= 0)
    _m = minn(Src0 - C0, Zero) * C1
    op1 = add(
        "KAN_CUBE1",
        Spec(body=sq(_m) * _m,
             reference=lambda in0, s0, s1: (np.minimum(in0 - s0, 0) * s1) ** 3),
        rd1=False,
    )
    # out = in1 - min(in0 - C0, 0)^3           (C0=2, in1=op1 out -> 6*basis)
    _m2 = minn(Src0 - C0, Zero)
    op2 = add(
        "KAN_CUBE2C",
        Spec(body=Src1 - sq(_m2) * _m2,
             reference=lambda in0, in1, s0: in1 - np.minimum(in0 - s0, 0) ** 3),
        rd1=True,
    )
    return op1, op2


_BUILD_CACHE = {}


def _build(repeat=1):
    """Build + finalize the per-core Bass program. Cached per repeat count."""
    if repeat in _BUILD_CACHE:
        return _BUILD_CACHE[repeat]
    OP1, OP2 = _register_ops()

    nc = bacc.Bacc("TRN2", target_bir_lowering=False, debug=False,
                   enable_asserts=False, num_devices=NCORES)

    x_d = nc.dram_tensor("x", [S, F], _f32, kind="ExternalInput")
    wsp_d = nc.dram_tensor("wsp", [FT, J, 128, U], _f16, kind="ExternalInput")
    wb_d = nc.dram_tensor("wb", [FT, 128, U], _f16, kind="ExternalInput")
    sa_d = nc.dram_tensor("sa", [128, FT], _f32, kind="ExternalInput")      # 1.5*gamma
    ba_d = nc.dram_tensor("ba", [128, FT * J], _f32, kind="ExternalInput")  # 1.5*beta+4.5-(j+2)
    gsc_d = nc.dram_tensor("gsc", [128, FT], _f32, kind="ExternalInput")    # gamma
    gbi_d = nc.dram_tensor("gbi", [128, FT], _f32, kind="ExternalInput")    # beta
    out_d = nc.dram_tensor("out", [S, U], _f32, kind="ExternalOutput")

    with TileContext(nc) as tc:
        with (
            tc.tile_pool(name="consts", bufs=1) as consts,
            tc.tile_pool(name="xbuf", bufs=1) as xbuf,
            tc.tile_pool(name="stats", bufs=4) as statsp,
            tc.tile_pool(name="xn", bufs=2) as xnp,
            tc.tile_pool(name="silu", bufs=2) as silup,
            tc.tile_pool(name="abuf", bufs=4) as abufp,
            tc.tile_pool(name="t1buf", bufs=4) as t1p,
            tc.tile_pool(name="bases", bufs=2) as basesp,
            tc.tile_pool(name="tpsum", bufs=2, space="PSUM") as tpsum,
            tc.tile_pool(name="opsum", bufs=4, space="PSUM") as opsum,
        ):
            ident = consts.tile([128, 128], _f32)
            make_identity(nc, ident)
            eps_t = consts.tile([128, 1], _f32)
            nc.vector.memset(eps_t, LN_EPS)

            wsp_t = [[consts.tile([128, U], _f16, name=f"wsp{ft}_{j}", tag=f"wsp{ft}_{j}")
                      for j in range(J)] for ft in range(FT)]
            wb_t = [consts.tile([128, U], _f16, name=f"wb{ft}", tag=f"wb{ft}")
                    for ft in range(FT)]
            for ft in range(FT):
                nc.sync.dma_start(out=wb_t[ft][:], in_=wb_d[ft])
                for j in range(J):
                    nc.sync.dma_start(out=wsp_t[ft][j][:], in_=wsp_d[ft, j])
            sa_t = consts.tile([128, FT], _f32)
            ba_t = consts.tile([128, FT * J], _f32)
            gsc_t = consts.tile([128, FT], _f32)
            gbi_t = consts.tile([128, FT], _f32)
            nc.sync.dma_start(out=sa_t[:], in_=sa_d[:])
            nc.sync.dma_start(out=ba_t[:], in_=ba_d[:])
            nc.sync.dma_start(out=gsc_t[:], in_=gsc_d[:])
            nc.sync.dma_start(out=gbi_t[:], in_=gbi_d[:])

            x_all = xbuf.tile([128, ST, F], _f32)          # all 16 s-tiles resident
            mv_all = consts.tile([128, 2 * ST], _f32)      # interleaved mean/var
            std_all = consts.tile([128, ST], _f32)
            rstd_all = consts.tile([128, ST], _f32)

            for r in range(repeat):
                # ---- phase 1: load x, LayerNorm statistics -------------------
                for st in range(ST):
                    nc.sync.dma_start(out=x_all[:, st, :], in_=x_d[st * 128:(st + 1) * 128, :])
                    bn6 = statsp.tile([128, 6], _f32, tag="bn6")
                    nc.vector.bn_stats(out=bn6[:], in_=x_all[:, st, :])
                    nc.vector.bn_aggr(out=mv_all[:, 2 * st:2 * st + 2], in_=bn6[:])
                var_cols = mv_all[:].rearrange("p (s two) -> p s two", two=2)[:, :, 1]
                nc.scalar.activation(out=std_all[:], in_=var_cols,
                                     func=mybir.ActivationFunctionType.Sqrt,
                                     bias=eps_t[:], scale=1.0)
                nc.vector.reciprocal(out=rstd_all[:], in_=std_all[:])

                # ---- phases 2+3: per 512-token chunk -------------------------
                for c in range(NCH):
                    xn_t = []
                    for lt in range(LT):
                        st = c * LT + lt
                        xn = xnp.tile([128, F], _f32, tag=f"xn{lt}")
                        nc.vector.tensor_scalar(
                            out=xn[:], in0=x_all[:, st, :],
                            scalar1=mv_all[:, 2 * st:2 * st + 1],
                            scalar2=rstd_all[:, st:st + 1],
                            op0=mybir.AluOpType.subtract,
                            op1=mybir.AluOpType.mult,
                        )
                        xn_t.append(xn)

                    silu_t = []
                    bases_t = [[None] * J for _ in range(FT)]
                    for ft in range(FT):
                        pt = tpsum.tile([128, CHUNK], _f32, tag="pt")
                        for lt in range(LT):
                            nc.tensor.transpose(
                                out=pt[:, lt * 128:(lt + 1) * 128],
                                in_=xn_t[lt][:, ft * 128:(ft + 1) * 128],
                                identity=ident[:],
                            )
                        sl = silup.tile([128, CHUNK], _f16, tag=f"silu{ft}")
                        nc.scalar.activation(out=sl[:], in_=pt[:],
                                             func=mybir.ActivationFunctionType.Silu,
                                             bias=gbi_t[:, ft:ft + 1],
                                             scale=gsc_t[:, ft:ft + 1])
                        silu_t.append(sl)
                        for j in range(J):
                            a = abufp.tile([128, CHUNK], _f16, tag="a")
                            nc.scalar.activation(out=a[:], in_=pt[:],
                                                 func=mybir.ActivationFunctionType.Abs,
                                                 bias=ba_t[:, ft * J + j:ft * J + j + 1],
                                                 scale=sa_t[:, ft:ft + 1])
                            t1 = t1p.tile([128, CHUNK], _f16, tag="t1")
                            nc.vector._custom_dve(OP1, out=t1[:], in0=a[:],
                                                  s0=1.0, s1=CBRT4)
                            bs = basesp.tile([128, CHUNK], _f16, tag=f"b{ft}_{j}")
                            nc.vector._custom_dve(
                                OP2, out=bs[:], in0=a[:],
                                in1=t1[:].rearrange("p (a b) -> p a b", a=1),
                                s0=2.0)
                            bases_t[ft][j] = bs

                    for lt in range(LT):
                        ps = opsum.tile([128, U], _f32, tag="ps")
                        first = True
                        n_mm = FT + FT * J
                        k = 0
                        for ft in range(FT):
                            k += 1
                            nc.tensor.matmul(ps[:], silu_t[ft][:, lt * 128:(lt + 1) * 128],
                                             wb_t[ft][:], start=first, stop=(k == n_mm))
                            first = False
                        for ft in range(FT):
                            for j in range(J):
                                k += 1
                                nc.tensor.matmul(ps[:],
                                                 bases_t[ft][j][:, lt * 128:(lt + 1) * 128],
                                                 wsp_t[ft][j][:],
                                                 start=False, stop=(k == n_mm))
                        ob = silup.tile([128, U], _f32, tag="ob", bufs=4)
                        nc.scalar.copy(out=ob[:], in_=ps[:])
                        row = c * CHUNK + lt * 128
                        nc.sync.dma_start(out=out_d[row:row + 128, :], in_=ob[:])

    nc.finalize()
    _BUILD_CACHE[repeat] = nc
    return nc


def _prep_inputs(x, grid, base_weight, base_bias, spline_weight, ln_gamma, ln_beta):
    x = np.ascontiguousarray(np.asarray(x), dtype=np.float32)
    bw = np.asarray(base_weight, dtype=np.float32)
    sw = np.asarray(spline_weight, dtype=np.float32).reshape(U, F, J) / 6.0
    gam = np.asarray(ln_gamma, dtype=np.float32)
    bet = np.asarray(ln_beta, dtype=np.float32)

    # wsp[ft, j, p, u] = sw[u, 128*ft+p, j] / 6
    wsp = np.ascontiguousarray(
        sw.transpose(1, 2, 0).reshape(FT, 128, J, U).transpose(0, 2, 1, 3),
        dtype=np.float16)
    # wb[ft, p, u] = bw[u, 128*ft+p]
    wb = np.ascontiguousarray(bw.T.reshape(FT, 128, U), dtype=np.float16)

    gam_pf = gam.reshape(FT, 128).T            # [128, FT]
    bet_pf = bet.reshape(FT, 128).T
    sa = np.ascontiguousarray(1.5 * gam_pf, dtype=np.float32)
    jj = np.arange(J, dtype=np.float32)
    # ba[p, ft*J+j] = 1.5*beta + 4.5 - (j+2)
    ba = np.ascontiguousarray(
        (1.5 * bet_pf[:, :, None] + (4.5 - (jj + 2.0))[None, None, :]).reshape(128, FT * J),
        dtype=np.float32)
    gsc = np.ascontiguousarray(gam_pf, dtype=np.float32)
    gbi = np.ascontiguousarray(bet_pf, dtype=np.float32)

    common = {"wsp": wsp, "wb": wb, "sa": sa, "ba": ba, "gsc": gsc, "gbi": gbi}
    in_maps = [{"x": np.ascontiguousarray(x[b]), **common} for b in range(NCORES)]
    return in_maps


_LAST_RESULTS = None


def run(inputs, repeat=1, trace=False):
    """Run the SPMD kernel; returns the full [B, S, U] fp32 output."""
    global _LAST_RESULTS
    nc = _build(repeat)
    in_maps = _prep_inputs(**inputs)
    res = run_bass_kernel_spmd(nc, in_maps, core_ids=list(range(NCORES)), trace=trace)
    _LAST_RESULTS = res
    bias = np.asarray(inputs["base_bias"], dtype=np.float32)
    out = np.stack([res.results[b]["out"] for b in range(NCORES)], axis=0)
    if bias.any():
        out = out + bias[None, None, :]
    return out.astype(np.float32)


def kernel(**inputs):
    return run(inputs, repeat=1)


# revision 7
# speedup vs baseline: 1065.2237x; 1065.2237x over previous
# kernel.py — Trainium2 Bass kernel for nn_KANLinear (LayerNorm -> silu/base + cubic
# B-spline/spline matmuls), data-parallel over the batch dim across 8 NeuronCores.
#
# Math (per core, x: [S=2048, F=512]):
#   xn   = LayerNorm(x) * gamma + beta          (eps = 1e-3)
#   out  = silu(xn) @ base_weight.T + bases(xn) @ spline_weight.T + base_bias
# where bases are the 6 cubic B-spline basis functions on the uniform grid
# [-3, 3] (h = 2/3). Closed form used on-device (verified exact vs Cox-de Boor):
#   u     = 1.5*xn + 4.5
#   a_j   = |u - (j+2)|                 j = 0..5
#   6*B_j = (cbrt4*min(a_j-1,0))^3 - min(a_j-2,0)^3
# The 1/6 is folded into the spline weights host-side; gamma/beta are folded
# into the per-partition scale/bias of the Abs/Silu activation passes.
#
# Engine mapping per core:
#   DVE   : bn_stats/bn_aggr LN stats, (x-mu)*rstd, 2 custom DVE cube ops per basis
#   ACT   : sqrt(var+eps), |scale*xn+bias| passes, Silu
#   PE    : 64 transposes of xn ([s,f] -> [f,s]) + 448 fp16 matmuls
#   DMA   : x in, fp16 weights in, PSUM -> DRAM out
# Output accumulates base+spline matmuls in PSUM in natural [s, u] layout.

import os
import numpy as np

import concourse.bacc as bacc
import concourse.mybir as mybir
from concourse.tile import TileContext
from concourse.bass_utils import run_bass_kernel_spmd
from concourse.masks import make_identity
from concourse.dve_spec import Spec, Src0, Src1, C0, C1, Zero, minn, sq, lower
from concourse.dve_ops import DveOp, OPS
import concourse.dve_ops as dve_ops_mod
from concourse.dve_uop import DveOpSpec

B, S, F, U, J = 8, 2048, 512, 512, 6
NCORES = 8
FT = F // 128          # 4 f-tiles
ST = S // 128          # 16 s-tiles
CHUNK = 512            # tokens per processing chunk
NCH = S // CHUNK       # 4 chunks
LT = CHUNK // 128      # 4 s-subtiles per chunk
CBRT4 = float(4.0 ** (1.0 / 3.0))
LN_EPS = 1e-3

_f32 = mybir.dt.float32
_f16 = mybir.dt.float16


def _register_ops():
    """Register the two custom DVE ops (idempotent)."""
    existing = {op.name for op in OPS}

    def add(name, spec, rd1):
        if name in existing:
            return next(op for op in OPS if op.name == name)
        shas = {}
        for ver in ("v3",):
            s = DveOpSpec(name=name, opcode=0, uops=lower(spec, ver=ver), rd1_en=rd1)
            shas[ver] = s.sha(ver)
        op = DveOp(name, spec, subdim=False, uops_sha=shas)
        OPS.append(op)
        dve_ops_mod.CUSTOM_DVE_SPECS[op.name] = op.spec
        dve_ops_mod._SUB_OPCODE_FOR_NAME[op.name] = (
            dve_ops_mod._CUSTOM_DVE_ROW_BASE + len(OPS) - 1
        )
        return op

    # out = (C1 * min(in0 - C0, 0))^3          (C0=1, C1=cbrt4  -> 4*m1^3, <= 0)
    _m = minn(Src0 - C0, Zero) * C1
    op1 = add(
        "KAN_CUBE1",
        Spec(body=sq(_m) * _m,
             reference=lambda in0, s0, s1: (np.minimum(in0 - s0, 0) * s1) ** 3),
        rd1=False,
    )
    # out = in1 - min(in0 - C0, 0)^3           (C0=2, in1=op1 out -> 6*basis)
    _m2 = minn(Src0 - C0, Zero)
    op2 = add(
        "KAN_CUBE2C",
        Spec(body=Src1 - sq(_m2) * _m2,
             reference=lambda in0, in1, s0: in1 - np.minimum(in0 - s0, 0) ** 3),
        rd1=True,
    )
    return op1, op2


_BUILD_CACHE = {}


def _build(repeat=1):
    """Build + finalize the per-core Bass program. Cached per repeat count."""
    if repeat in _BUILD_CACHE:
        return _BUILD_CACHE[repeat]
    OP1, OP2 = _register_ops()

    nc = bacc.Bacc("TRN2", target_bir_lowering=False, debug=False,
                   enable_asserts=False, num_devices=NCORES)

    x_d = nc.dram_tensor("x", [S, F], _f32, kind="ExternalInput")
    wsp_d = nc.dram_tensor("wsp", [FT, J, 128, U], _f16, kind="ExternalInput")
    wb_d = nc.dram_tensor("wb", [FT, 128, U], _f16, kind="ExternalInput")
    sa_d = nc.dram_tensor("sa", [128, FT], _f32, kind="ExternalInput")      # 1.5*gamma
    ba_d = nc.dram_tensor("ba", [128, FT * J], _f32, kind="ExternalInput")  # 1.5*beta+4.5-(j+2)
    gsc_d = nc.dram_tensor("gsc", [128, FT], _f32, kind="ExternalInput")    # gamma
    gbi_d = nc.dram_tensor("gbi", [128, FT], _f32, kind="ExternalInput")    # beta
    out_d = nc.dram_tensor("out", [S, U], _f32, kind="ExternalOutput")

    with TileContext(nc) as tc:
        with (
            tc.tile_pool(name="consts", bufs=1) as consts,
            tc.tile_pool(name="xbuf", bufs=1) as xbuf,
            tc.tile_pool(name="stats", bufs=4) as statsp,
            tc.tile_pool(name="xn", bufs=2) as xnp,
            tc.tile_pool(name="silu", bufs=2) as silup,
            tc.tile_pool(name="abuf", bufs=4) as abufp,
            tc.tile_pool(name="t1buf", bufs=4) as t1p,
            tc.tile_pool(name="bases", bufs=2) as basesp,
            tc.tile_pool(name="tpsum", bufs=2, space="PSUM") as tpsum,
            tc.tile_pool(name="opsum", bufs=4, space="PSUM") as opsum,
        ):
            ident = consts.tile([128, 128], _f32)
            make_identity(nc, ident)
            eps_t = consts.tile([128, 1], _f32)
            nc.vector.memset(eps_t, LN_EPS)

            wsp_t = [[consts.tile([128, U], _f16, name=f"wsp{ft}_{j}", tag=f"wsp{ft}_{j}")
                      for j in range(J)] for ft in range(FT)]
            wb_t = [consts.tile([128, U], _f16, name=f"wb{ft}", tag=f"wb{ft}")
                    for ft in range(FT)]
            for ft in range(FT):
                nc.sync.dma_start(out=wb_t[ft][:], in_=wb_d[ft])
                for j in range(J):
                    nc.sync.dma_start(out=wsp_t[ft][j][:], in_=wsp_d[ft, j])
            sa_t = consts.tile([128, FT], _f32)
            ba_t = consts.tile([128, FT * J], _f32)
            gsc_t = consts.tile([128, FT], _f32)
            gbi_t = consts.tile([128, FT], _f32)
            nc.sync.dma_start(out=sa_t[:], in_=sa_d[:])
            nc.sync.dma_start(out=ba_t[:], in_=ba_d[:])
            nc.sync.dma_start(out=gsc_t[:], in_=gsc_d[:])
            nc.sync.dma_start(out=gbi_t[:], in_=gbi_d[:])

            x_all = xbuf.tile([128, ST, F], _f32)          # all 16 s-tiles resident
            mv_all = consts.tile([128, 2 * ST], _f32)      # interleaved mean/var
            std_all = consts.tile([128, ST], _f32)
            rstd_all = consts.tile([128, ST], _f32)

            import contextlib

            loop_cm = tc.For_i(0, repeat, 1) if repeat > 1 else contextlib.nullcontext()
            with loop_cm:
                # ---- phase 1: load x, LayerNorm statistics -------------------
                for st in range(ST):
                    nc.sync.dma_start(out=x_all[:, st, :], in_=x_d[st * 128:(st + 1) * 128, :])
                    bn6 = statsp.tile([128, 6], _f32, tag="bn6")
                    nc.vector.bn_stats(out=bn6[:], in_=x_all[:, st, :])
                    nc.vector.bn_aggr(out=mv_all[:, 2 * st:2 * st + 2], in_=bn6[:])
                var_cols = mv_all[:].rearrange("p (s two) -> p s two", two=2)[:, :, 1]
                nc.scalar.activation(out=std_all[:], in_=var_cols,
                                     func=mybir.ActivationFunctionType.Sqrt,
                                     bias=eps_t[:], scale=1.0)
                nc.vector.reciprocal(out=rstd_all[:], in_=std_all[:])

                # ---- phases 2+3: per 512-token chunk -------------------------
                for c in range(NCH):
                    xn_t = []
                    for lt in range(LT):
                        st = c * LT + lt
                        xn = xnp.tile([128, F], _f32, tag=f"xn{lt}")
                        nc.vector.tensor_scalar(
                            out=xn[:], in0=x_all[:, st, :],
                            scalar1=mv_all[:, 2 * st:2 * st + 1],
                            scalar2=rstd_all[:, st:st + 1],
                            op0=mybir.AluOpType.subtract,
                            op1=mybir.AluOpType.mult,
                        )
                        xn_t.append(xn)

                    silu_t = []
                    bases_t = [[None] * J for _ in range(FT)]
                    for ft in range(FT):
                        pt = tpsum.tile([128, CHUNK], _f32, tag="pt")
                        for lt in range(LT):
                            nc.tensor.transpose(
                                out=pt[:, lt * 128:(lt + 1) * 128],
                                in_=xn_t[lt][:, ft * 128:(ft + 1) * 128],
                                identity=ident[:],
                            )
                        sl = silup.tile([128, CHUNK], _f16, tag=f"silu{ft}")
                        nc.scalar.activation(out=sl[:], in_=pt[:],
                                             func=mybir.ActivationFunctionType.Silu,
                                             bias=gbi_t[:, ft:ft + 1],
                                             scale=gsc_t[:, ft:ft + 1])
                        silu_t.append(sl)
                        for j in range(J):
                            a = abufp.tile([128, CHUNK], _f16, tag="a")
                            nc.scalar.activation(out=a[:], in_=pt[:],
                                                 func=mybir.ActivationFunctionType.Abs,
                                                 bias=ba_t[:, ft * J + j:ft * J + j + 1],
                                                 scale=sa_t[:, ft:ft + 1])
                            t1 = t1p.tile([128, CHUNK], _f16, tag="t1")
                            nc.vector._custom_dve(OP1, out=t1[:], in0=a[:],
                                                  s0=1.0, s1=CBRT4)
                            bs = basesp.tile([128, CHUNK], _f16, tag=f"b{ft}_{j}")
                            nc.vector._custom_dve(
                                OP2, out=bs[:], in0=a[:],
                                in1=t1[:].rearrange("p (a b) -> p a b", a=1),
                                s0=2.0)
                            bases_t[ft][j] = bs

                    for lt in range(LT):
                        ps = opsum.tile([128, U], _f32, tag="ps")
                        first = True
                        n_mm = FT + FT * J
                        k = 0
                        for ft in range(FT):
                            k += 1
                            nc.tensor.matmul(ps[:], silu_t[ft][:, lt * 128:(lt + 1) * 128],
                                             wb_t[ft][:], start=first, stop=(k == n_mm))
                            first = False
                        for ft in range(FT):
                            for j in range(J):
                                k += 1
                                nc.tensor.matmul(ps[:],
                                                 bases_t[ft][j][:, lt * 128:(lt + 1) * 128],
                                                 wsp_t[ft][j][:],
                                                 start=False, stop=(k == n_mm))
                        ob = silup.tile([128, U], _f32, tag="ob", bufs=4)
                        nc.scalar.copy(out=ob[:], in_=ps[:])
                        row = c * CHUNK + lt * 128
                        nc.sync.dma_start(out=out_d[row:row + 128, :], in_=ob[:])

    nc.finalize()
    _BUILD_CACHE[repeat] = nc
    return nc


def _prep_inputs(x, grid, base_weight, base_bias, spline_weight, ln_gamma, ln_beta):
    x = np.ascontiguousarray(np.asarray(x), dtype=np.float32)
    bw = np.asarray(base_weight, dtype=np.float32)
    sw = np.asarray(spline_weight, dtype=np.float32).reshape(U, F, J) / 6.0
    gam = np.asarray(ln_gamma, dtype=np.float32)
    bet = np.asarray(ln_beta, dtype=np.float32)

    # wsp[ft, j, p, u] = sw[u, 128*ft+p, j] / 6
    wsp = np.ascontiguousarray(
        sw.transpose(1, 2, 0).reshape(FT, 128, J, U).transpose(0, 2, 1, 3),
        dtype=np.float16)
    # wb[ft, p, u] = bw[u, 128*ft+p]
    wb = np.ascontiguousarray(bw.T.reshape(FT, 128, U), dtype=np.float16)

    gam_pf = gam.reshape(FT, 128).T            # [128, FT]
    bet_pf = bet.reshape(FT, 128).T
    sa = np.ascontiguousarray(1.5 * gam_pf, dtype=np.float32)
    jj = np.arange(J, dtype=np.float32)
    # ba[p, ft*J+j] = 1.5*beta + 4.5 - (j+2)
    ba = np.ascontiguousarray(
        (1.5 * bet_pf[:, :, None] + (4.5 - (jj + 2.0))[None, None, :]).reshape(128, FT * J),
        dtype=np.float32)
    gsc = np.ascontiguousarray(gam_pf, dtype=np.float32)
    gbi = np.ascontiguousarray(bet_pf, dtype=np.float32)

    common = {"wsp": wsp, "wb": wb, "sa": sa, "ba": ba, "gsc": gsc, "gbi": gbi}
    in_maps = [{"x": np.ascontiguousarray(x[b]), **common} for b in range(NCORES)]
    return in_maps


_LAST_RESULTS = None


def run(inputs, repeat=1, trace=False):
    """Run the SPMD kernel; returns the full [B, S, U] fp32 output."""
    global _LAST_RESULTS
    nc = _build(repeat)
    in_maps = _prep_inputs(**inputs)
    res = run_bass_kernel_spmd(nc, in_maps, core_ids=list(range(NCORES)), trace=trace)
    _LAST_RESULTS = res
    bias = np.asarray(inputs["base_bias"], dtype=np.float32)
    out = np.stack([res.results[b]["out"] for b in range(NCORES)], axis=0)
    if bias.any():
        out = out + bias[None, None, :]
    return out.astype(np.float32)


def kernel(**inputs):
    return run(inputs, repeat=1)


# revision 8
# speedup vs baseline: 1287.2809x; 1.2085x over previous
# kernel.py — Trainium2 Bass kernel for nn_KANLinear (LayerNorm -> silu/base + cubic
# B-spline/spline matmuls), data-parallel over the batch dim across 8 NeuronCores.
#
# Math (per core, x: [S=2048, F=512]):
#   xn   = LayerNorm(x) * gamma + beta          (eps = 1e-3)
#   out  = silu(xn) @ base_weight.T + bases(xn) @ spline_weight.T + base_bias
# where bases are the 6 cubic B-spline basis functions on the uniform grid
# [-3, 3] (h = 2/3). Closed form used on-device (verified exact vs Cox-de Boor):
#   u     = 1.5*xn + 4.5
#   a_j   = |u - (j+2)|                 j = 0..5
#   6*B_j = (cbrt4*min(a_j-1,0))^3 - min(a_j-2,0)^3
# The 1/6 is folded into the spline weights host-side; gamma/beta are folded
# into the per-partition scale/bias of the Abs/Silu activation passes.
#
# Engine mapping per core:
#   DVE   : bn_stats/bn_aggr LN stats, (x-mu)*rstd, 2 custom DVE cube ops per basis
#   ACT   : sqrt(var+eps), |scale*xn+bias| passes, Silu
#   PE    : 64 transposes of xn ([s,f] -> [f,s]) + 448 fp16 matmuls
#   DMA   : x in, fp16 weights in, PSUM -> DRAM out
# Output accumulates base+spline matmuls in PSUM in natural [s, u] layout.

import os
import numpy as np

import concourse.bacc as bacc
import concourse.mybir as mybir
from concourse.tile import TileContext
from concourse.bass_utils import run_bass_kernel_spmd
from concourse.masks import make_identity
from concourse.dve_spec import Spec, Src0, Src1, C0, C1, Zero, minn, sq, lower
from concourse.dve_ops import DveOp, OPS
import concourse.dve_ops as dve_ops_mod
from concourse.dve_uop import DveOpSpec

B, S, F, U, J = 8, 2048, 512, 512, 6
NCORES = 8
FT = F // 128          # 4 f-tiles
ST = S // 128          # 16 s-tiles
CHUNK = 512            # tokens per processing chunk
NCH = S // CHUNK       # 4 chunks
LT = CHUNK // 128      # 4 s-subtiles per chunk
CBRT4 = float(4.0 ** (1.0 / 3.0))
LN_EPS = 1e-3

_f32 = mybir.dt.float32
_f16 = mybir.dt.float16


def _register_ops():
    """Register the two custom DVE ops (idempotent)."""
    existing = {op.name for op in OPS}

    def add(name, spec, rd1):
        if name in existing:
            return next(op for op in OPS if op.name == name)
        shas = {}
        for ver in ("v3",):
            s = DveOpSpec(name=name, opcode=0, uops=lower(spec, ver=ver), rd1_en=rd1)
            shas[ver] = s.sha(ver)
        op = DveOp(name, spec, subdim=False, uops_sha=shas)
        OPS.append(op)
        dve_ops_mod.CUSTOM_DVE_SPECS[op.name] = op.spec
        dve_ops_mod._SUB_OPCODE_FOR_NAME[op.name] = (
            dve_ops_mod._CUSTOM_DVE_ROW_BASE + len(OPS) - 1
        )
        return op

    # out = (C1 * min(in0 - C0, 0))^3          (C0=1, C1=cbrt4  -> 4*m1^3, <= 0)
    _m = minn(Src0 - C0, Zero) * C1
    op1 = add(
        "KAN_CUBE1",
        Spec(body=sq(_m) * _m,
             reference=lambda in0, s0, s1: (np.minimum(in0 - s0, 0) * s1) ** 3),
        rd1=False,
    )
    # out = in1 - min(in0 - C0, 0)^3           (C0=2, in1=op1 out -> 6*basis)
    _m2 = minn(Src0 - C0, Zero)
    op2 = add(
        "KAN_CUBE2C",
        Spec(body=Src1 - sq(_m2) * _m2,
             reference=lambda in0, in1, s0: in1 - np.minimum(in0 - s0, 0) ** 3),
        rd1=True,
    )
    return op1, op2


_BUILD_CACHE = {}


def _build(repeat=1):
    """Build + finalize the per-core Bass program. Cached per repeat count."""
    if repeat in _BUILD_CACHE:
        return _BUILD_CACHE[repeat]
    OP1, OP2 = _register_ops()

    nc = bacc.Bacc("TRN2", target_bir_lowering=False, debug=False,
                   enable_asserts=False, num_devices=NCORES)

    x_d = nc.dram_tensor("x", [S, F], _f32, kind="ExternalInput")
    wsp_d = nc.dram_tensor("wsp", [FT, J, 128, U], _f16, kind="ExternalInput")
    wb_d = nc.dram_tensor("wb", [FT, 128, U], _f16, kind="ExternalInput")
    sa_d = nc.dram_tensor("sa", [128, FT], _f32, kind="ExternalInput")      # 1.5*gamma
    ba_d = nc.dram_tensor("ba", [128, FT * J], _f32, kind="ExternalInput")  # 1.5*beta+4.5-(j+2)
    gsc_d = nc.dram_tensor("gsc", [128, FT], _f32, kind="ExternalInput")    # gamma
    gbi_d = nc.dram_tensor("gbi", [128, FT], _f32, kind="ExternalInput")    # beta
    out_d = nc.dram_tensor("out", [S, U], _f32, kind="ExternalOutput")

    with TileContext(nc) as tc:
        with (
            tc.tile_pool(name="consts", bufs=1) as consts,
            tc.tile_pool(name="xbuf", bufs=1) as xbuf,
            tc.tile_pool(name="stats", bufs=4) as statsp,
            tc.tile_pool(name="xn", bufs=2) as xnp,
            tc.tile_pool(name="silu", bufs=2) as silup,
            tc.tile_pool(name="abuf", bufs=8) as abufp,
            tc.tile_pool(name="t1buf", bufs=8) as t1p,
            tc.tile_pool(name="bases", bufs=2) as basesp,
            tc.tile_pool(name="tpsum", bufs=3, space="PSUM") as tpsum,
            tc.tile_pool(name="opsum", bufs=4, space="PSUM") as opsum,
        ):
            ident = consts.tile([128, 128], _f32)
            make_identity(nc, ident)
            eps_t = consts.tile([128, 1], _f32)
            nc.vector.memset(eps_t, LN_EPS)

            wsp_t = [[consts.tile([128, U], _f16, name=f"wsp{ft}_{j}", tag=f"wsp{ft}_{j}")
                      for j in range(J)] for ft in range(FT)]
            wb_t = [consts.tile([128, U], _f16, name=f"wb{ft}", tag=f"wb{ft}")
                    for ft in range(FT)]
            for ft in range(FT):
                nc.sync.dma_start(out=wb_t[ft][:], in_=wb_d[ft])
                for j in range(J):
                    nc.sync.dma_start(out=wsp_t[ft][j][:], in_=wsp_d[ft, j])
            sa_t = consts.tile([128, FT], _f32)
            ba_t = consts.tile([128, FT * J], _f32)
            gsc_t = consts.tile([128, FT], _f32)
            gbi_t = consts.tile([128, FT], _f32)
            nc.sync.dma_start(out=sa_t[:], in_=sa_d[:])
            nc.sync.dma_start(out=ba_t[:], in_=ba_d[:])
            nc.sync.dma_start(out=gsc_t[:], in_=gsc_d[:])
            nc.sync.dma_start(out=gbi_t[:], in_=gbi_d[:])

            x_all = xbuf.tile([128, ST, F], _f32)          # all 16 s-tiles resident
            mv_all = consts.tile([128, 2 * ST], _f32)      # interleaved mean/var
            std_all = consts.tile([128, ST], _f32)
            rstd_all = consts.tile([128, ST], _f32)

            import contextlib

            loop_cm = tc.For_i(0, repeat, 1) if repeat > 1 else contextlib.nullcontext()
            with loop_cm:
                # ---- phase 1: load x, LayerNorm statistics -------------------
                for st in range(ST):
                    nc.sync.dma_start(out=x_all[:, st, :], in_=x_d[st * 128:(st + 1) * 128, :])
                    bn6 = statsp.tile([128, 6], _f32, tag="bn6")
                    nc.vector.bn_stats(out=bn6[:], in_=x_all[:, st, :])
                    nc.vector.bn_aggr(out=mv_all[:, 2 * st:2 * st + 2], in_=bn6[:])
                var_cols = mv_all[:].rearrange("p (s two) -> p s two", two=2)[:, :, 1]
                nc.scalar.activation(out=std_all[:], in_=var_cols,
                                     func=mybir.ActivationFunctionType.Sqrt,
                                     bias=eps_t[:], scale=1.0)
                nc.vector.reciprocal(out=rstd_all[:], in_=std_all[:])

                # ---- phases 2+3: per 512-token chunk -------------------------
                for c in range(NCH):
                    xn_t = []
                    for lt in range(LT):
                        st = c * LT + lt
                        xn = xnp.tile([128, F], _f32, tag=f"xn{lt}")
                        nc.vector.tensor_scalar(
                            out=xn[:], in0=x_all[:, st, :],
                            scalar1=mv_all[:, 2 * st:2 * st + 1],
                            scalar2=rstd_all[:, st:st + 1],
                            op0=mybir.AluOpType.subtract,
                            op1=mybir.AluOpType.mult,
                        )
                        xn_t.append(xn)

                    silu_t = []
                    bases_t = [[None] * J for _ in range(FT)]
                    for ft in range(FT):
                        pt = tpsum.tile([128, CHUNK], _f32, tag="pt")
                        for lt in range(LT):
                            nc.tensor.transpose(
                                out=pt[:, lt * 128:(lt + 1) * 128],
                                in_=xn_t[lt][:, ft * 128:(ft + 1) * 128],
                                identity=ident[:],
                            )
                        sl = silup.tile([128, CHUNK], _f16, tag=f"silu{ft}")
                        nc.scalar.activation(out=sl[:], in_=pt[:],
                                             func=mybir.ActivationFunctionType.Silu,
                                             bias=gbi_t[:, ft:ft + 1],
                                             scale=gsc_t[:, ft:ft + 1])
                        silu_t.append(sl)
                        for j in range(J):
                            a = abufp.tile([128, CHUNK], _f16, tag="a")
                            nc.scalar.activation(out=a[:], in_=pt[:],
                                                 func=mybir.ActivationFunctionType.Abs,
                                                 bias=ba_t[:, ft * J + j:ft * J + j + 1],
                                                 scale=sa_t[:, ft:ft + 1])
                            t1 = t1p.tile([128, CHUNK], _f16, tag="t1")
                            nc.vector._custom_dve(OP1, out=t1[:], in0=a[:],
                                                  s0=1.0, s1=CBRT4)
                            bs = basesp.tile([128, CHUNK], _f16, tag=f"b{ft}_{j}")
                            nc.vector._custom_dve(
                                OP2, out=bs[:], in0=a[:],
                                in1=t1[:].rearrange("p (a b) -> p a b", a=1),
                                s0=2.0)
                            bases_t[ft][j] = bs

                    for lt in range(LT):
                        ps = opsum.tile([128, U], _f32, tag="ps")
                        first = True
                        n_mm = FT + FT * J
                        k = 0
                        for ft in range(FT):
                            k += 1
                            nc.tensor.matmul(ps[:], silu_t[ft][:, lt * 128:(lt + 1) * 128],
                                             wb_t[ft][:], start=first, stop=(k == n_mm))
                            first = False
                        for ft in range(FT):
                            for j in range(J):
                                k += 1
                                nc.tensor.matmul(ps[:],
                                                 bases_t[ft][j][:, lt * 128:(lt + 1) * 128],
                                                 wsp_t[ft][j][:],
                                                 start=False, stop=(k == n_mm))
                        ob = silup.tile([128, U], _f32, tag="ob", bufs=4)
                        nc.scalar.copy(out=ob[:], in_=ps[:])
                        row = c * CHUNK + lt * 128
                        nc.sync.dma_start(out=out_d[row:row + 128, :], in_=ob[:])

    nc.finalize()
    _BUILD_CACHE[repeat] = nc
    return nc


def _prep_inputs(x, grid, base_weight, base_bias, spline_weight, ln_gamma, ln_beta):
    x = np.ascontiguousarray(np.asarray(x), dtype=np.float32)
    bw = np.asarray(base_weight, dtype=np.float32)
    sw = np.asarray(spline_weight, dtype=np.float32).reshape(U, F, J) / 6.0
    gam = np.asarray(ln_gamma, dtype=np.float32)
    bet = np.asarray(ln_beta, dtype=np.float32)

    # wsp[ft, j, p, u] = sw[u, 128*ft+p, j] / 6
    wsp = np.ascontiguousarray(
        sw.transpose(1, 2, 0).reshape(FT, 128, J, U).transpose(0, 2, 1, 3),
        dtype=np.float16)
    # wb[ft, p, u] = bw[u, 128*ft+p]
    wb = np.ascontiguousarray(bw.T.reshape(FT, 128, U), dtype=np.float16)

    gam_pf = gam.reshape(FT, 128).T            # [128, FT]
    bet_pf = bet.reshape(FT, 128).T
    sa = np.ascontiguousarray(1.5 * gam_pf, dtype=np.float32)
    jj = np.arange(J, dtype=np.float32)
    # ba[p, ft*J+j] = 1.5*beta + 4.5 - (j+2)
    ba = np.ascontiguousarray(
        (1.5 * bet_pf[:, :, None] + (4.5 - (jj + 2.0))[None, None, :]).reshape(128, FT * J),
        dtype=np.float32)
    gsc = np.ascontiguousarray(gam_pf, dtype=np.float32)
    gbi = np.ascontiguousarray(bet_pf, dtype=np.float32)

    common = {"wsp": wsp, "wb": wb, "sa": sa, "ba": ba, "gsc": gsc, "gbi": gbi}
    in_maps = [{"x": np.ascontiguousarray(x[b]), **common} for b in range(NCORES)]
    return in_maps


_LAST_RESULTS = None


def run(inputs, repeat=1, trace=False):
    """Run the SPMD kernel; returns the full [B, S, U] fp32 output."""
    global _LAST_RESULTS
    nc = _build(repeat)
    in_maps = _prep_inputs(**inputs)
    res = run_bass_kernel_spmd(nc, in_maps, core_ids=list(range(NCORES)), trace=trace)
    _LAST_RESULTS = res
    bias = np.asarray(inputs["base_bias"], dtype=np.float32)
    out = np.stack([res.results[b]["out"] for b in range(NCORES)], axis=0)
    if bias.any():
        out = out + bias[None, None, :]
    return out.astype(np.float32)


def kernel(**inputs):
    return run(inputs, repeat=1)
